# revision 1
# baseline (speedup 1.0000x reference)
"""Trainium2 Bass kernel for nn_CrossAttentionLayer (B=8, N=2048, Q=256, D=1024, H=16).

Strategy: data-parallel over batch (1 sample per NeuronCore, 8 cores).
Per-core, everything is expressed as matmuls in fp32r (TF32-like, 4x faster
than fp32 on the PE) except the probability @ V leg which runs in bf16.

Host-side preprocessing (cheap numpy):
  - transpose sources/queries/weights so contraction dims land on SBUF
    partitions without any on-device transposes
  - fold the V bias through the output projection (softmax rows sum to 1):
      out = attn @ (X_v + 1 b_v^T) @ W_o^T + b_out + queries
          = attn @ X_v @ W_o^T + (b_out + W_o b_v) + queries
  - drop the K bias entirely (adds a per-query constant to scores ->
    softmax invariant)
  - fold the 1/sqrt(HD) scale and b_q into the Q projection eviction

Device phases per core:
  P0  DMA loads (sourcesT resident in fp32r)
  P1  V = sources @ W_v^T           -> bf16, heads padded with a ones column
                                       (gives softmax denominators for free)
  P2  kT = (sources @ W_k^T)^T      -> fp32r  [D, N]
  P3  qT = ((queries @ W_q^T)+b_q)/8^T -> fp32r [D, Q]
  P4  per head: scoresT = kT_h^T-slices x qT_h  [N, Q] -> exp (ACT, bf16)
      -> outT_h[65, Q] = [V_h | 1]^T @ expT (accumulated over N tiles)
      -> normalize rows 0..63 by reciprocal of row 64 (PE-broadcast)
  P5  out = attnoutT^T @ W_o^T + (queries + b_out + W_o b_v), DMA out
"""

import numpy as np
from contextlib import ExitStack

import concourse.bass as bass
import concourse.mybir as mybir
import concourse.tile as tile
from concourse import bacc
from concourse.bass_utils import run_bass_kernel_spmd

F32 = mybir.dt.float32
F32R = mybir.dt.float32r
BF16 = mybir.dt.bfloat16
AF = mybir.ActivationFunctionType

B, N, Q, D, H = 8, 2048, 256, 1024, 16
N_CORES = 8


def build(N=N, Q=Q, D=D, H=H):
    HD = D // H           # head dim (64)
    KT = D // 128         # contraction (din) tiles
    MT = D // 128         # output (dout) tiles
    NT = N // 128         # source-token tiles
    QT = Q // 128         # query-token tiles
    HPT = 128 // HD       # heads per 128-row dout tile (2)
    NCH = min(512, D)     # fp32r moving-dim chunk (<= one PSUM bank)
    CH = 4                # score n-tiles per exp chunk ([128, CH*Q] <= 2 banks)
    KBLK = min(1024, N)   # kT eviction block
    assert D % NCH == 0 and N % (CH * 128) == 0 and N % KBLK == 0 and Q <= 512

    nc = bacc.Bacc(None, target_bir_lowering=False)
    srcT = nc.declare_dram_parameter("srcT", [D, N], F32R, isOutput=False)
    qryT = nc.declare_dram_parameter("qryT", [D, Q], F32R, isOutput=False)
    wvT = nc.declare_dram_parameter("wvT", [D, D], F32R, isOutput=False)
    wkT = nc.declare_dram_parameter("wkT", [D, D], F32R, isOutput=False)
    wqT = nc.declare_dram_parameter("wqT", [D, D], F32R, isOutput=False)
    woT = nc.declare_dram_parameter("woT", [D, D], F32R, isOutput=False)
    bq = nc.declare_dram_parameter("bq", [D], F32, isOutput=False)
    resid = nc.declare_dram_parameter("resid", [Q, D], F32, isOutput=False)
    out = nc.declare_dram_parameter("out", [Q, D], F32, isOutput=True)

    with tile.TileContext(nc) as tc, ExitStack() as ctx:
        psum = ctx.enter_context(tc.tile_pool(name="psum", bufs=4, space="PSUM"))
        kt_pool = ctx.enter_context(tc.tile_pool(name="ktp", bufs=1))
        v_pool = ctx.enter_context(tc.tile_pool(name="vp", bufs=1))
        qt_pool = ctx.enter_context(tc.tile_pool(name="qtp", bufs=1))

        kt_sb = kt_pool.tile([128, MT, N], F32R)
        v_sb = v_pool.tile([128, NT, H, HD + 1], BF16)
        qt_sb = qt_pool.tile([128, MT, Q], F32R)

        with ExitStack() as pctx:
            src_pool = pctx.enter_context(tc.tile_pool(name="srcp", bufs=1))
            wbig_pool = pctx.enter_context(tc.tile_pool(name="wbig", bufs=1))
            wsm_pool = pctx.enter_context(tc.tile_pool(name="wsm", bufs=2))
            qry_pool = pctx.enter_context(tc.tile_pool(name="qryp", bufs=1))

            src_sb = src_pool.tile([128, KT, N], F32R)
            srcT_r = srcT.rearrange("(kt p) n -> kt p n", p=128)
            for k in range(KT):
                nc.sync.dma_start(out=src_sb[:, k, :], in_=srcT_r[k])

            # ---- P1: V projection -> bf16, [n, h, hd(+ones)] ----
            HPC = NCH // HD  # heads per dout chunk
            nc.vector.memset(v_sb[:, :, :, HD:HD + 1], 1.0)
            for c in range(D // NCH):
                wv_c = wbig_pool.tile([128, KT, NCH], F32R, tag="wbig")
                nc.scalar.dma_start(
                    out=wv_c,
                    in_=wvT.rearrange("(kt p) d -> p kt d", p=128)[:, :, c * NCH:(c + 1) * NCH],
                )
                for t in range(NT):
                    ps = psum.tile([128, NCH], F32, tag="ps")
                    for k in range(KT):
                        nc.tensor.matmul(
                            ps[:],
                            lhsT=src_sb[:, k, t * 128:(t + 1) * 128],
                            rhs=wv_c[:, k, :],
                            start=(k == 0), stop=(k == KT - 1),
                        )
                    nc.vector.tensor_copy(
                        out=v_sb[:, t, c * HPC:(c + 1) * HPC, 0:HD],
                        in_=ps[:].rearrange("p (h d) -> p h d", h=HPC),
                    )

            # ---- P2: K projection -> kT [dout, n] fp32r ----
            for m in range(MT):
                wk_m = wsm_pool.tile([128, KT, 128], F32R, tag="wsm")
                nc.scalar.dma_start(
                    out=wk_m,
                    in_=wkT.rearrange("(kt p) d -> p kt d", p=128)[:, :, m * 128:(m + 1) * 128],
                )
                for half in range(N // KBLK):
                    ps = psum.tile([128, KBLK], F32, tag="ps")
                    for k in range(KT):
                        for c in range(KBLK // 512):
                            nc.tensor.matmul(
                                ps[:, c * 512:(c + 1) * 512],
                                lhsT=wk_m[:, k, :],
                                rhs=src_sb[:, k, half * KBLK + c * 512: half * KBLK + (c + 1) * 512],
                                start=(k == 0), stop=(k == KT - 1),
                            )
                    nc.vector.tensor_copy(
                        out=kt_sb[:, m, half * KBLK:(half + 1) * KBLK], in_=ps
                    )

            # ---- P3: Q projection -> qT [dout, q] fp32r, (x + b_q)/sqrt(HD) ----
            qry_sb = qry_pool.tile([128, KT, Q], F32R, tag="qry")
            nc.scalar.dma_start(out=qry_sb, in_=qryT.rearrange("(kt p) q -> p kt q", p=128))
            bq_sb = qry_pool.tile([128, MT], F32, tag="bq")
            nc.scalar.dma_start(out=bq_sb, in_=bq.rearrange("(mt p) -> p mt", p=128))
            for m in range(MT):
                wq_m = wsm_pool.tile([128, KT, 128], F32R, tag="wsm")
                nc.scalar.dma_start(
                    out=wq_m,
                    in_=wqT.rearrange("(kt p) d -> p kt d", p=128)[:, :, m * 128:(m + 1) * 128],
                )
                ps = psum.tile([128, Q], F32, tag="ps")
                for k in range(KT):
                    nc.tensor.matmul(
                        ps[:], lhsT=wq_m[:, k, :], rhs=qry_sb[:, k, :],
                        start=(k == 0), stop=(k == KT - 1),
                    )
                nc.vector.tensor_scalar(
                    out=qt_sb[:, m, :], in0=ps[:],
                    scalar1=bq_sb[:, m:m + 1], scalar2=1.0 / np.sqrt(HD),
                    op0=mybir.AluOpType.add, op1=mybir.AluOpType.mult,
                )

        # ---- P4: attention per head ----
        with ExitStack() as actx:
            exp_pool = actx.enter_context(tc.tile_pool(name="expp", bufs=3))
            rc_pool = actx.enter_context(tc.tile_pool(name="rcp", bufs=2))
            ao_pool = actx.enter_context(tc.tile_pool(name="aop", bufs=1))
            one_pool = actx.enter_context(tc.tile_pool(name="onep", bufs=1))
            wo_pool = actx.enter_context(tc.tile_pool(name="wop", bufs=1))
            res_pool = actx.enter_context(tc.tile_pool(name="resp", bufs=1))
            out_pool = actx.enter_context(tc.tile_pool(name="outp", bufs=2))

            ones_f32 = one_pool.tile([1, HD], F32, tag="ones32")
            nc.vector.memset(ones_f32, 1.0)
            ones_sb = one_pool.tile([1, HD], F32R, tag="ones")
            nc.vector.tensor_copy(ones_sb, ones_f32)

            ao_sb = ao_pool.tile([128, MT, Q], F32R)

            def emit_scores(h, expt):
                mt, po = divmod(h, HPT)
                po *= HD
                for chk in range(NT // CH):
                    ps = psum.tile([128, CH, Q], F32, tag="ps", name=f"ps_s{h}_{chk}")
                    for j in range(CH):
                        nt = chk * CH + j
                        nc.tensor.matmul(
                            ps[:, j, :],
                            lhsT=kt_sb[po:po + HD, mt, nt * 128:(nt + 1) * 128],
                            rhs=qt_sb[po:po + HD, mt, :],
                            start=True, stop=True,
                        )
                    nc.scalar.activation(
                        out=expt[:, chk * CH:(chk + 1) * CH, :], in_=ps[:], func=AF.Exp
                    )

            def emit_attn(h, expt):
                mt, po = divmod(h, HPT)
                po *= HD
                pso = psum.tile([HD + 1, Q], F32, tag="ps", name=f"pso{h}")
                for nt in range(NT):
                    nc.tensor.matmul(
                        pso[:], lhsT=v_sb[:, nt, h, :], rhs=expt[:, nt, :],
                        start=(nt == 0), stop=(nt == NT - 1),
                    )
                # normalize: rows 0..HD-1 divided by row HD (the ones-column sum)
                rc32 = rc_pool.tile([1, Q], F32, tag="rc32", name=f"rc32_{h}")
                nc.vector.reciprocal(rc32, pso[HD:HD + 1, :])
                rc = rc_pool.tile([1, Q], F32R, tag="rc", name=f"rc{h}")
                nc.vector.tensor_copy(rc, rc32)
                rbp = psum.tile([HD, Q], F32, tag="ps", name=f"rbp{h}")
                nc.tensor.matmul(rbp[:], lhsT=ones_sb[:], rhs=rc[:], start=True, stop=True)
                rb = rc_pool.tile([HD, Q], F32, tag="rb", name=f"rb{h}")
                nc.vector.tensor_copy(rb, rbp)
                nc.vector.tensor_mul(ao_sb[po:po + HD, mt, :], pso[0:HD, :], rb[:])

            # software pipeline: scores/exp of head h overlap attn@V of h-1,
            # so the PE never stalls on the ACT exp round-trip
            expts = {}
            for h in range(H):
                expts[h] = exp_pool.tile([128, NT, Q], BF16, tag="exp", name=f"expt{h}")
                emit_scores(h, expts[h])
                if h > 0:
                    emit_attn(h - 1, expts[h - 1])
            emit_attn(H - 1, expts[H - 1])

            # ---- P5: output projection + residual ----
            wo_sb = wo_pool.tile([128, KT, D], F32R, tag="wo")
            nc.sync.dma_start(out=wo_sb, in_=woT.rearrange("(kt p) d -> p kt d", p=128))
            res_sb = res_pool.tile([128, QT, D], F32, tag="res")
            nc.sync.dma_start(out=res_sb, in_=resid.rearrange("(qt p) d -> p qt d", p=128))
            for qt in range(QT):
                ps = psum.tile([128, D], F32, tag="ps")
                for k in range(KT):
                    for c in range(D // NCH):
                        nc.tensor.matmul(
                            ps[:, c * NCH:(c + 1) * NCH],
                            lhsT=ao_sb[:, k, qt * 128:(qt + 1) * 128],
                            rhs=wo_sb[:, k, c * NCH:(c + 1) * NCH],
                            start=(k == 0), stop=(k == KT - 1),
                        )
                osb = out_pool.tile([128, D], F32, tag="osb")
                nc.vector.tensor_add(osb[:], ps[:], res_sb[:, qt, :])
                nc.sync.dma_start(out=out[qt * 128:(qt + 1) * 128, :], in_=osb)

    nc.finalize()
    return nc


_NC_CACHE = {}


def _get_nc():
    key = (N, Q, D, H)
    if key not in _NC_CACHE:
        _NC_CACHE[key] = build()
    return _NC_CACHE[key]


def make_in_maps(sources, queries, w_in, b_in, w_out, b_out):
    sources = np.asarray(sources, dtype=np.float32)
    queries = np.asarray(queries, dtype=np.float32)
    w_in = np.asarray(w_in, dtype=np.float32)
    b_in = np.asarray(b_in, dtype=np.float32)
    w_out = np.asarray(w_out, dtype=np.float32)
    b_out = np.asarray(b_out, dtype=np.float32)

    w_q, w_k, w_v = w_in[0:D], w_in[D:2 * D], w_in[2 * D:3 * D]
    b_q, b_v = b_in[0:D], b_in[2 * D:3 * D]
    # b_k dropped: constant shift along softmax axis
    wqT = np.ascontiguousarray(w_q.T)
    wkT = np.ascontiguousarray(w_k.T)
    wvT = np.ascontiguousarray(w_v.T)
    woT = np.ascontiguousarray(w_out.T)
    bout_eff = b_out + w_out @ b_v

    in_maps = []
    for b in range(B):
        in_maps.append({
            "srcT": np.ascontiguousarray(sources[b].T),
            "qryT": np.ascontiguousarray(queries[b].T),
            "wvT": wvT, "wkT": wkT, "wqT": wqT, "woT": woT,
            "bq": b_q,
            "resid": queries[b] + bout_eff[None, :],
        })
    return in_maps


def kernel(sources, queries, w_in, b_in, w_out, b_out, _trace=False):
    nc = _get_nc()
    in_maps = make_in_maps(sources, queries, w_in, b_in, w_out, b_out)
    res = run_bass_kernel_spmd(nc, in_maps, core_ids=list(range(N_CORES)), trace=_trace)
    out = np.stack([res.results[b]["out"] for b in range(B)], axis=0)
    if _trace:
        kernel.last_exec_time_ns = res.exec_time_ns
        kernel.last_results = res
    return out



# revision 12
# speedup vs baseline: 1.8634x; 1.8634x over previous
"""Trainium2 Bass kernel for nn_CrossAttentionLayer (B=8, N=2048, Q=256, D=1024, H=16).

Data-parallel over batch: 1 sample per NeuronCore, 8 cores, no collectives.

Math identities (host-side folds):
  - b_k dropped (constant shift along the softmax axis)
  - b_v folded through out_proj: bout_eff = b_out + W_o b_v (softmax rows sum to 1)
  - b_q and the 1/sqrt(HD) scale folded into the Q-projection eviction
  - exp computed as exp(s - 3): the e^-3 cancels in softmax normalization and
    keeps probs inside fp8-e4m3 normal range (max score ~5.5 -> e^2.5 ~ 12)

Precision plan (rel-err gate is 2e-2; residual dominates the output):
  - K/V/Q/out projections in fp8 e4m3 (weights host-scaled x32, unscaled at
    psum eviction); K and V use DoubleRow (2 k-tiles per matmul ~ 2x PE rate)
  - scores in bf16 with per-head-pair ROW TILING: head 2p on PE rows 0-63,
    head 2p+1 on rows 64-127, running concurrently (contraction = HD = 64)
  - probs in fp8 from ACT exp; attn@V in fp8 DoubleRow with a ones column
    appended to V (row 64 of the psum = softmax denominator, free)
  - normalization: batched reciprocal of denominators + PE broadcast (x32 to
    re-center fp8) + one DVE mul per pair

Engine budget per core: PE ~103us, ACT (exp only) ~73us, DVE (all psum
evictions) ~55us, DMA ~8.3MB in. ACT table + HAM warm-up during initial DMA.
"""

import numpy as np
import ml_dtypes
from contextlib import ExitStack

import concourse.bass as bass
import concourse.mybir as mybir
import concourse.tile as tile
from concourse import bacc
from concourse.bass_utils import run_bass_kernel_spmd

F32 = mybir.dt.float32
BF16 = mybir.dt.bfloat16
FP8 = mybir.dt.float8e4
AF = mybir.ActivationFunctionType
ALU = mybir.AluOpType
DR = mybir.MatmulPerfMode.DoubleRow

NP_FP8 = ml_dtypes.float8_e4m3
NP_BF16 = ml_dtypes.bfloat16

B, N, Q, D, H = 8, 2048, 256, 1024, 16
HD = D // H            # 64
KT = D // 128          # 8 contraction tiles
MT = D // 128          # 8 output tiles
NT = N // 128          # 16 token tiles
NW = N // 512          # 4 token windows (DMA + Kproj chunking)
PAIRS = H // 2         # 8 head pairs
WS = 32.0              # host weight pre-scale for fp8
N_CORES = 8


def build():
    nc = bacc.Bacc(None, target_bir_lowering=False)
    src8 = nc.declare_dram_parameter("src8", [NW, 128, KT, 512], FP8, isOutput=False)
    qry8 = nc.declare_dram_parameter("qry8", [128, KT, Q], FP8, isOutput=False)
    wk8 = nc.declare_dram_parameter("wk8", [128, KT, D], FP8, isOutput=False)
    wv8 = nc.declare_dram_parameter("wv8", [128, KT, D], FP8, isOutput=False)
    wq8 = nc.declare_dram_parameter("wq8", [128, KT, D], FP8, isOutput=False)
    wo8 = nc.declare_dram_parameter("wo8", [128, KT, D], FP8, isOutput=False)
    bq8 = nc.declare_dram_parameter("bq8", [128, MT], F32, isOutput=False)
    resid = nc.declare_dram_parameter("resid", [128, Q // 128, D], F32, isOutput=False)
    out = nc.declare_dram_parameter("out", [Q, D], F32, isOutput=True)

    with tile.TileContext(nc) as tc, ExitStack() as ctx:
        proj_ps = ctx.enter_context(tc.tile_pool(name="projps", bufs=2, space="PSUM"))
        sc_ps = ctx.enter_context(tc.tile_pool(name="scps", bufs=2, space="PSUM"))
        pso_ps = ctx.enter_context(tc.tile_pool(name="psops", bufs=2, space="PSUM"))

        big = ctx.enter_context(tc.tile_pool(name="big", bufs=1))
        exp_pool = ctx.enter_context(tc.tile_pool(name="expp", bufs=4))
        rb_pool = ctx.enter_context(tc.tile_pool(name="rbp", bufs=2))
        out_pool = ctx.enter_context(tc.tile_pool(name="outp", bufs=2))

        src_sb = big.tile([128, KT, NW, 512], FP8, tag="src")
        wk_sb = big.tile([128, KT, D], FP8, tag="wk")
        wv_sb = big.tile([128, KT, D], FP8, tag="wv")
        wq_sb = big.tile([128, KT, D], FP8, tag="wq")
        wo_sb = big.tile([128, KT, D], FP8, tag="wo")
        qry_sb = big.tile([128, KT, Q], FP8, tag="qry")
        bq_sb = big.tile([128, MT], F32, tag="bq")
        kt_sb = big.tile([128, MT, N], BF16, tag="kt")
        qt_sb = big.tile([128, MT, Q], BF16, tag="qt")
        v_sb = big.tile([128, NT, H, 66], FP8, tag="v")
        den_sb = big.tile([1, H, Q], F32, tag="den")
        rcp_sb = big.tile([1, H, Q], F32, tag="rcp")
        rc16_sb = big.tile([1, H, Q], BF16, tag="rc16")
        e32_sb = big.tile([1, 64], BF16, tag="e32")
        ao_sb = big.tile([128, MT, Q], BF16, tag="ao")
        ao8_sb = big.tile([128, MT, Q], FP8, tag="ao8")
        resid_sb = big.tile([128, Q // 128, D], F32, tag="res")
        warm_sb = big.tile([16, 128], BF16, tag="warm")
        warmo_sb = big.tile([16, 16], F32, tag="warmo")
        nb_sb = big.tile([128, 1], F32, tag="negbias")

        # ---- init: memsets (gpsimd), ACT exp-table warm, PE HAM warm ----
        nc.gpsimd.memset(warm_sb, 0.0)
        nc.vector.memset(warmo_sb, 0.0)
        nc.vector.memset(e32_sb, WS)  # broadcast row (x32 fp8 re-center for ao8)
        nc.gpsimd.memset(v_sb[:, :, :, 64:65], 1.0)   # denominator ones column
        nc.gpsimd.memset(v_sb[:, :, :, 65:66], 0.0)   # padding
        nc.gpsimd.memset(nb_sb, -3.0)                 # exp re-centering bias
        # load the exp table set early (hides the ~2.7us ACT_TABLE_LOAD)
        nc.scalar.activation(out=warmo_sb[0:1, :], in_=warm_sb[0:1, 0:16], func=AF.Exp)
        # dummy matmuls to trip the PE HAM un-throttle during the DMA window
        for i in range(24):
            wp = proj_ps.tile([16, 128], F32, tag="proj", name=f"warm{i}")
            nc.tensor.matmul(wp[:], lhsT=warm_sb[:, 0:16], rhs=warm_sb[:], start=True, stop=True)

        # ---- DMA loads (sync queue), priority order ----
        nc.sync.dma_start(out=qry_sb, in_=qry8[:])
        nc.sync.dma_start(out=wq_sb, in_=wq8[:])
        nc.sync.dma_start(out=bq_sb, in_=bq8[:])
        for w in range(NW):
            nc.sync.dma_start(out=src_sb[:, :, w, :], in_=src8[w])
        nc.sync.dma_start(out=wk_sb, in_=wk8[:])
        nc.sync.dma_start(out=wv_sb, in_=wv8[:])
        nc.sync.dma_start(out=wo_sb, in_=wo8[:])
        nc.sync.dma_start(out=resid_sb, in_=resid[:])

        # ---- Q projection (plain fp8): qt = (psum/(WS*8)) + b_q/8 ----
        for m in range(MT):
            qp = proj_ps.tile([128, Q], F32, tag="proj", name=f"qp{m}")
            for k in range(KT):
                nc.tensor.matmul(
                    qp[:], lhsT=wq_sb[:, k, m * 128:(m + 1) * 128],
                    rhs=qry_sb[:, k, :], start=(k == 0), stop=(k == KT - 1),
                )
            nc.vector.tensor_scalar(
                out=qt_sb[:, m, :], in0=qp[:],
                scalar1=1.0 / (WS * 8.0), scalar2=bq_sb[:, m:m + 1],
                op0=ALU.mult, op1=ALU.add,
            )

        # ---- K projection (fp8 DoubleRow): kT[dout, tok] = W_k @ src^T ----
        def emit_kproj(m, w):
            kp = proj_ps.tile([128, 512], F32, tag="proj", name=f"kp{m}_{w}")
            for k in range(KT // 2):
                nc.tensor.matmul(
                    kp[:],
                    lhsT=wk_sb[:, 2 * k:2 * k + 2, m * 128:(m + 1) * 128],
                    rhs=src_sb[:, 2 * k:2 * k + 2, w, :],
                    start=(k == 0), stop=(k == KT // 2 - 1), perf_mode=DR,
                )
            nc.vector.tensor_scalar_mul(
                out=kt_sb[:, m, w * 512:(w + 1) * 512], in0=kp[:], scalar1=1.0 / WS
            )

        # ---- V projection (fp8 DoubleRow): v[tok, h, hd] = src @ W_v^T ----
        def emit_vproj(c, t):
            vp = proj_ps.tile([128, 512], F32, tag="proj", name=f"vp{c}_{t}")
            for k in range(KT // 2):
                nc.tensor.matmul(
                    vp[:],
                    lhsT=src_sb[:, 2 * k:2 * k + 2, t // 4, (t % 4) * 128:(t % 4) * 128 + 128],
                    rhs=wv_sb[:, 2 * k:2 * k + 2, c * 512:(c + 1) * 512],
                    start=(k == 0), stop=(k == KT // 2 - 1), perf_mode=DR,
                )
            nc.vector.tensor_scalar_mul(
                out=v_sb[:, t, c * 8:(c + 1) * 8, 0:64],
                in0=vp[:].rearrange("p (h d) -> p h d", h=8),
                scalar1=1.0 / WS,
            )

        for m in range(4):
            for w in range(NW):
                emit_kproj(m, w)
        for t in range(NT):
            emit_vproj(0, t)
        for m in range(4, MT):
            for w in range(NW):
                emit_kproj(m, w)
        for t in range(NT):
            emit_vproj(1, t)

        # ---- attention: per head pair, row-tiled scores + DR attn@V ----
        def emit_normalize(plo, phi):
            nc.vector.reciprocal(
                rcp_sb[:, 2 * plo:2 * phi + 2, :], den_sb[:, 2 * plo:2 * phi + 2, :]
            )
            nc.vector.tensor_copy(
                rc16_sb[:, 2 * plo:2 * phi + 2, :], rcp_sb[:, 2 * plo:2 * phi + 2, :]
            )
            for p in range(plo, phi + 1):
                rbp = proj_ps.tile([128, Q], F32, tag="proj", name=f"rbp{p}")
                for par in range(2):
                    nc.tensor.matmul(
                        rbp[par * 64:par * 64 + 64, :], lhsT=e32_sb[:],
                        rhs=rc16_sb[:, 2 * p + par, :], start=True, stop=True,
                    )
                rb = rb_pool.tile([128, Q], BF16, tag="rb", name=f"rb{p}")
                nc.vector.tensor_copy(rb, rbp)
                nc.vector.tensor_mul(ao8_sb[:, p, :], ao_sb[:, p, :], rb[:])

        for p in range(PAIRS):
            expt = {}
            for par in range(2):  # 0 = head 2p (rows 0-63), 1 = head 2p+1 (rows 64-127)
                expt[par] = exp_pool.tile([128, NT, Q], FP8, tag="exp", name=f"expt{p}_{par}")
            for c in range(NT // 4):
                sc = {}
                for par in range(2):
                    sc[par] = sc_ps.tile([128, 4, Q], F32, tag="sc", name=f"sc{p}_{c}_{par}")
                for j in range(4):
                    nt = 4 * c + j
                    for par in range(2):
                        po = par * 64
                        nc.tensor.matmul(
                            sc[par][:, j, :],
                            lhsT=kt_sb[po:po + 64, p, nt * 128:(nt + 1) * 128],
                            rhs=qt_sb[po:po + 64, p, :],
                            start=True, stop=True,
                        )
                for par in range(2):
                    nc.scalar.activation(
                        out=expt[par][:, 4 * c:4 * c + 4, :], in_=sc[par][:],
                        func=AF.Exp, bias=nb_sb[:],
                    )
            for par in range(2):
                h = 2 * p + par
                pso = pso_ps.tile([65, Q], F32, tag="pso", name=f"pso{h}")
                for tt in range(NT // 2):
                    nc.tensor.matmul(
                        pso[:],
                        lhsT=v_sb[:, 2 * tt:2 * tt + 2, h, 0:65],
                        rhs=expt[par][:, 2 * tt:2 * tt + 2, :],
                        start=(tt == 0), stop=(tt == NT // 2 - 1), perf_mode=DR,
                    )
                nc.vector.tensor_copy(ao_sb[par * 64:par * 64 + 64, p, :], pso[0:64, :])
                nc.vector.tensor_copy(den_sb[:, h, :], pso[64:65, :])
            if p == 3:
                emit_normalize(0, 3)
        emit_normalize(4, 7)

        # ---- out projection (fp8 DoubleRow) + residual ----
        for qt in range(Q // 128):
            for c in range(2):
                op = proj_ps.tile([128, 512], F32, tag="proj", name=f"op{qt}_{c}")
                for m in range(MT // 2):
                    nc.tensor.matmul(
                        op[:],
                        lhsT=ao8_sb[:, 2 * m:2 * m + 2, qt * 128:(qt + 1) * 128],
                        rhs=wo_sb[:, 2 * m:2 * m + 2, c * 512:(c + 1) * 512],
                        start=(m == 0), stop=(m == MT // 2 - 1), perf_mode=DR,
                    )
                ot = out_pool.tile([128, 512], BF16, tag="ot", name=f"ot{qt}_{c}")
                nc.scalar.activation(out=ot[:], in_=op[:], func=AF.Copy, scale=1.0 / (WS * WS))
                of = out_pool.tile([128, 512], F32, tag="of", name=f"of{qt}_{c}")
                nc.vector.tensor_add(of[:], ot[:], resid_sb[:, qt, c * 512:(c + 1) * 512])
                nc.sync.dma_start(
                    out=out[qt * 128:(qt + 1) * 128, c * 512:(c + 1) * 512], in_=of
                )

    nc.finalize()
    return nc


_NC_CACHE = {}


def _get_nc():
    if "nc" not in _NC_CACHE:
        _NC_CACHE["nc"] = build()
    return _NC_CACHE["nc"]


def _fp8(x):
    return np.clip(x, -240.0, 240.0).astype(NP_FP8)


def make_in_maps(sources, queries, w_in, b_in, w_out, b_out):
    sources = np.asarray(sources, dtype=np.float32)
    queries = np.asarray(queries, dtype=np.float32)
    w_in = np.asarray(w_in, dtype=np.float32)
    b_in = np.asarray(b_in, dtype=np.float32)
    w_out = np.asarray(w_out, dtype=np.float32)
    b_out = np.asarray(b_out, dtype=np.float32)

    w_q, w_k, w_v = w_in[0:D], w_in[D:2 * D], w_in[2 * D:3 * D]
    b_q, b_v = b_in[0:D], b_in[2 * D:3 * D]
    bout_eff = b_out + w_out @ b_v

    def wprep(w):  # [dout, din] -> fp8 [128, KT, D] p-major of (w.T * WS)
        wt = np.ascontiguousarray(w.T) * WS
        return _fp8(wt.reshape(KT, 128, D).transpose(1, 0, 2))

    wk8 = wprep(w_k)
    wv8 = wprep(w_v)
    wq8 = wprep(w_q)
    wo8 = wprep(w_out)
    bq8 = (b_q / 8.0).reshape(MT, 128).transpose(1, 0).copy()

    in_maps = []
    for b in range(B):
        st = sources[b].T  # [D, N]
        src8 = _fp8(st.reshape(KT, 128, NW, 512).transpose(2, 1, 0, 3))
        qt = queries[b].T  # [D, Q]
        qry8 = _fp8(qt.reshape(KT, 128, Q).transpose(1, 0, 2))
        res = (queries[b] + bout_eff[None, :]).reshape(Q // 128, 128, D).transpose(1, 0, 2).copy()
        in_maps.append({
            "src8": src8, "qry8": qry8,
            "wk8": wk8, "wv8": wv8, "wq8": wq8, "wo8": wo8,
            "bq8": bq8, "resid": res,
        })
    return in_maps


def kernel(sources, queries, w_in, b_in, w_out, b_out, _trace=False):
    nc = _get_nc()
    in_maps = make_in_maps(sources, queries, w_in, b_in, w_out, b_out)
    res = run_bass_kernel_spmd(nc, in_maps, core_ids=list(range(N_CORES)), trace=_trace)
    out = np.stack([res.results[b]["out"] for b in range(B)], axis=0)
    if _trace:
        kernel.last_exec_time_ns = res.exec_time_ns
        kernel.last_results = res
    return out


# revision 15
# speedup vs baseline: 2.3998x; 1.2879x over previous
"""Trainium2 Bass kernel for nn_CrossAttentionLayer (B=8, N=2048, Q=256, D=1024, H=16).

Data-parallel over batch: 1 sample per NeuronCore, 8 cores, no collectives.

Math identities (host-side folds):
  - b_k dropped (constant shift along the softmax axis)
  - b_v folded through out_proj: bout_eff = b_out + W_o b_v (softmax rows sum to 1)
  - b_q and the 1/sqrt(HD) scale folded into the Q-projection eviction
  - exp computed as exp(s - 3): the e^-3 cancels in softmax normalization and
    keeps probs inside fp8-e4m3 normal range (max score ~5.5 -> e^2.5 ~ 12)

Precision plan (rel-err gate is 2e-2; residual dominates the output):
  - K/V/Q/out projections in fp8 e4m3 (weights host-scaled x32, unscaled at
    psum eviction); K and V use DoubleRow (2 k-tiles per matmul ~ 2x PE rate)
  - scores in bf16 with per-head-pair ROW TILING: head 2p on PE rows 0-63,
    head 2p+1 on rows 64-127, running concurrently (contraction = HD = 64)
  - probs in fp8 from ACT exp; attn@V in fp8 DoubleRow with a ones column
    appended to V (row 64 of the psum = softmax denominator, free)
  - normalization: batched reciprocal of denominators + PE broadcast (x32 to
    re-center fp8) + one DVE mul per pair

Engine budget per core: PE ~103us, ACT (exp only) ~73us, DVE (all psum
evictions) ~55us, DMA ~8.3MB in. ACT table + HAM warm-up during initial DMA.
"""

import numpy as np
import ml_dtypes
from contextlib import ExitStack

import concourse.bass as bass
import concourse.mybir as mybir
import concourse.tile as tile
from concourse import bacc
from concourse.bass_utils import run_bass_kernel_spmd

F32 = mybir.dt.float32
BF16 = mybir.dt.bfloat16
FP8 = mybir.dt.float8e4
AF = mybir.ActivationFunctionType
ALU = mybir.AluOpType
DR = mybir.MatmulPerfMode.DoubleRow

NP_FP8 = ml_dtypes.float8_e4m3
NP_BF16 = ml_dtypes.bfloat16

B, N, Q, D, H = 8, 2048, 256, 1024, 16
HD = D // H            # 64
KT = D // 128          # 8 contraction tiles
MT = D // 128          # 8 output tiles
NT = N // 128          # 16 token tiles
NW = N // 512          # 4 token windows (DMA + Kproj chunking)
PAIRS = H // 2         # 8 head pairs
WS = 32.0              # host weight pre-scale for fp8
N_CORES = 8


def build():
    nc = bacc.Bacc(None, target_bir_lowering=False)
    src8 = nc.declare_dram_parameter("src8", [NW, 128, KT, 512], FP8, isOutput=False)
    qry8 = nc.declare_dram_parameter("qry8", [128, KT, Q], FP8, isOutput=False)
    wk8 = nc.declare_dram_parameter("wk8", [128, KT, D], FP8, isOutput=False)
    wv8 = nc.declare_dram_parameter("wv8", [128, KT, D], FP8, isOutput=False)
    wq8 = nc.declare_dram_parameter("wq8", [128, KT, D], FP8, isOutput=False)
    wo8 = nc.declare_dram_parameter("wo8", [128, KT, D], FP8, isOutput=False)
    bq8 = nc.declare_dram_parameter("bq8", [128, MT], F32, isOutput=False)
    resid = nc.declare_dram_parameter("resid", [128, Q // 128, D], F32, isOutput=False)
    out = nc.declare_dram_parameter("out", [Q, D], F32, isOutput=True)

    with tile.TileContext(nc) as tc, ExitStack() as ctx:
        proj_ps = ctx.enter_context(tc.tile_pool(name="projps", bufs=2, space="PSUM"))
        sc_ps = ctx.enter_context(tc.tile_pool(name="scps", bufs=2, space="PSUM"))
        pso_ps = ctx.enter_context(tc.tile_pool(name="psops", bufs=2, space="PSUM"))

        big = ctx.enter_context(tc.tile_pool(name="big", bufs=1))
        exp_pool = ctx.enter_context(tc.tile_pool(name="expp", bufs=6))
        rb_pool = ctx.enter_context(tc.tile_pool(name="rbp", bufs=2))
        out_pool = ctx.enter_context(tc.tile_pool(name="outp", bufs=2))

        src_sb = big.tile([128, KT, NW, 512], FP8, tag="src")
        wk_sb = big.tile([128, KT, D], FP8, tag="wk")
        wv_sb = big.tile([128, KT, D], FP8, tag="wv")
        wq_sb = big.tile([128, KT, D], FP8, tag="wq")
        wo_sb = big.tile([128, KT, D], FP8, tag="wo")
        qry_sb = big.tile([128, KT, Q], FP8, tag="qry")
        bq_sb = big.tile([128, MT], F32, tag="bq")
        kt_sb = big.tile([128, MT, N], BF16, tag="kt")
        qt_sb = big.tile([128, MT, Q], BF16, tag="qt")
        v_sb = big.tile([128, NT, H, 66], FP8, tag="v")
        den_sb = big.tile([1, H, Q], BF16, tag="den")
        e32_sb = big.tile([1, 64], BF16, tag="e32")
        ao_sb = big.tile([128, MT, Q], BF16, tag="ao")
        ao8_sb = big.tile([128, MT, Q], FP8, tag="ao8")
        resid_sb = big.tile([128, Q // 128, D], F32, tag="res")
        warm_sb = big.tile([16, 128], BF16, tag="warm")
        warmo_sb = big.tile([16, 16], F32, tag="warmo")
        nb_sb = big.tile([128, 1], F32, tag="negbias")

        # ---- init: memsets (gpsimd), ACT exp-table warm, PE HAM warm ----
        nc.gpsimd.memset(warm_sb, 0.0)
        nc.vector.memset(warmo_sb, 0.0)
        nc.vector.memset(e32_sb, 1.0 / WS)  # denb = den/32 -> rb = 32/den
        nc.gpsimd.memset(v_sb[:, :, :, 64:65], 1.0)   # denominator ones column
        nc.gpsimd.memset(v_sb[:, :, :, 65:66], 0.0)   # padding
        nc.gpsimd.memset(nb_sb, -3.0)                 # exp re-centering bias
        # load the exp table set early (hides the ~2.7us ACT_TABLE_LOAD)
        nc.scalar.activation(out=warmo_sb[0:1, :], in_=warm_sb[0:1, 0:16], func=AF.Exp)
        # dummy matmuls to trip the PE HAM un-throttle during the DMA window
        for i in range(24):
            wp = proj_ps.tile([16, 128], F32, tag="proj", name=f"warm{i}")
            nc.tensor.matmul(wp[:], lhsT=warm_sb[:, 0:16], rhs=warm_sb[:], start=True, stop=True)

        # ---- DMA loads (sync queue), priority order ----
        nc.sync.dma_start(out=qry_sb, in_=qry8[:])
        nc.sync.dma_start(out=wq_sb, in_=wq8[:])
        nc.sync.dma_start(out=bq_sb, in_=bq8[:])
        for w in range(NW):
            nc.sync.dma_start(out=src_sb[:, :, w, :], in_=src8[w])
        nc.sync.dma_start(out=wk_sb, in_=wk8[:])
        nc.sync.dma_start(out=wv_sb, in_=wv8[:])
        nc.sync.dma_start(out=wo_sb, in_=wo8[:])
        nc.sync.dma_start(out=resid_sb, in_=resid[:])

        # ---- Q projection (plain fp8): qt = (psum/(WS*8)) + b_q/8 ----
        for m in range(MT):
            qp = proj_ps.tile([128, Q], F32, tag="proj", name=f"qp{m}")
            for k in range(KT):
                nc.tensor.matmul(
                    qp[:], lhsT=wq_sb[:, k, m * 128:(m + 1) * 128],
                    rhs=qry_sb[:, k, :], start=(k == 0), stop=(k == KT - 1),
                )
            nc.vector.tensor_scalar(
                out=qt_sb[:, m, :], in0=qp[:],
                scalar1=1.0 / (WS * 8.0), scalar2=bq_sb[:, m:m + 1],
                op0=ALU.mult, op1=ALU.add,
            )

        # ---- K projection (fp8 DoubleRow): kT[dout, tok] = W_k @ src^T ----
        def emit_kproj(m, w):
            kp = proj_ps.tile([128, 512], F32, tag="proj", name=f"kp{m}_{w}")
            for k in range(KT // 2):
                nc.tensor.matmul(
                    kp[:],
                    lhsT=wk_sb[:, 2 * k:2 * k + 2, m * 128:(m + 1) * 128],
                    rhs=src_sb[:, 2 * k:2 * k + 2, w, :],
                    start=(k == 0), stop=(k == KT // 2 - 1), perf_mode=DR,
                )
            nc.vector.tensor_scalar_mul(
                out=kt_sb[:, m, w * 512:(w + 1) * 512], in0=kp[:], scalar1=1.0 / WS
            )

        # ---- V projection (fp8 DoubleRow): v[tok, h, hd] = src @ W_v^T ----
        def emit_vproj(c, t):
            vp = proj_ps.tile([128, 512], F32, tag="proj", name=f"vp{c}_{t}")
            for k in range(KT // 2):
                nc.tensor.matmul(
                    vp[:],
                    lhsT=src_sb[:, 2 * k:2 * k + 2, t // 4, (t % 4) * 128:(t % 4) * 128 + 128],
                    rhs=wv_sb[:, 2 * k:2 * k + 2, c * 512:(c + 1) * 512],
                    start=(k == 0), stop=(k == KT // 2 - 1), perf_mode=DR,
                )
            nc.vector.tensor_scalar_mul(
                out=v_sb[:, t, c * 8:(c + 1) * 8, 0:64],
                in0=vp[:].rearrange("p (h d) -> p h d", h=8),
                scalar1=1.0 / WS,
            )

        # ---- attention: row-tiled scores + DR attn@V, emission-interleaved
        # with the K/V projections so ACT exp runs continuously ----
        expt = {}

        def emit_scores(p):
            for par in range(2):
                expt[(p, par)] = exp_pool.tile(
                    [128, NT, Q], FP8, tag="exp", name=f"expt{p}_{par}"
                )
            for c in range(NT // 4):
                sc = {}
                for par in range(2):
                    sc[par] = sc_ps.tile([128, 4, Q], F32, tag="sc", name=f"sc{p}_{c}_{par}")
                for j in range(4):
                    nt = 4 * c + j
                    for par in range(2):
                        po = par * 64
                        nc.tensor.matmul(
                            sc[par][:, j, :],
                            lhsT=kt_sb[po:po + 64, p, nt * 128:(nt + 1) * 128],
                            rhs=qt_sb[po:po + 64, p, :],
                            start=True, stop=True,
                        )
                for par in range(2):
                    nc.scalar.activation(
                        out=expt[(p, par)][:, 4 * c:4 * c + 4, :], in_=sc[par][:],
                        func=AF.Exp, bias=nb_sb[:],
                    )

        def emit_attnv(p):
            for par in range(2):
                h = 2 * p + par
                pso = pso_ps.tile([65, Q], F32, tag="pso", name=f"pso{h}")
                for tt in range(NT // 2):
                    nc.tensor.matmul(
                        pso[:],
                        lhsT=v_sb[:, 2 * tt:2 * tt + 2, h, 0:65],
                        rhs=expt[(p, par)][:, 2 * tt:2 * tt + 2, :],
                        start=(tt == 0), stop=(tt == NT // 2 - 1), perf_mode=DR,
                    )
                nc.vector.tensor_copy(ao_sb[par * 64:par * 64 + 64, p, :], pso[0:64, :])
                nc.vector.tensor_copy(den_sb[:, h, :], pso[64:65, :])

        def emit_norm(p):
            denb = proj_ps.tile([128, Q], F32, tag="proj", name=f"denb{p}")
            for par in range(2):
                nc.tensor.matmul(
                    denb[par * 64:par * 64 + 64, :], lhsT=e32_sb[:],
                    rhs=den_sb[:, 2 * p + par, :], start=True, stop=True,
                )
            rb = rb_pool.tile([128, Q], F32, tag="rb", name=f"rb{p}")
            nc.vector.reciprocal_approx_fast(out=rb[:], in_=denb[:])
            nc.vector.tensor_mul(ao8_sb[:, p, :], ao_sb[:, p, :], rb[:])

        emit_kproj(0, 0); emit_kproj(0, 1); emit_kproj(0, 2); emit_kproj(0, 3)
        for w in range(NW):
            emit_kproj(1, w)
        emit_scores(0)
        for t in range(NT):
            emit_vproj(0, t)
        for w in range(NW):
            emit_kproj(2, w)
        emit_scores(1)
        for w in range(NW):
            emit_kproj(3, w)
        emit_attnv(0)
        emit_scores(2)
        for w in range(NW):
            emit_kproj(4, w)
        emit_attnv(1)
        emit_scores(3)
        for t in range(8):
            emit_vproj(1, t)
        for w in range(NW):
            emit_kproj(5, w)
        emit_attnv(2)
        emit_scores(4)
        for t in range(8, NT):
            emit_vproj(1, t)
        for w in range(NW):
            emit_kproj(6, w)
        emit_attnv(3)
        emit_scores(5)
        emit_norm(0)
        emit_norm(1)
        for w in range(NW):
            emit_kproj(7, w)
        emit_attnv(4)
        emit_scores(6)
        emit_norm(2)
        emit_scores(7)
        emit_attnv(5)
        emit_norm(3)
        emit_attnv(6)
        emit_norm(4)
        emit_norm(5)
        emit_attnv(7)
        emit_norm(6)
        emit_norm(7)

        # ---- out projection (fp8 DoubleRow) + residual ----
        for qt in range(Q // 128):
            for c in range(2):
                op = proj_ps.tile([128, 512], F32, tag="proj", name=f"op{qt}_{c}")
                for m in range(MT // 2):
                    nc.tensor.matmul(
                        op[:],
                        lhsT=ao8_sb[:, 2 * m:2 * m + 2, qt * 128:(qt + 1) * 128],
                        rhs=wo_sb[:, 2 * m:2 * m + 2, c * 512:(c + 1) * 512],
                        start=(m == 0), stop=(m == MT // 2 - 1), perf_mode=DR,
                    )
                ot = out_pool.tile([128, 512], BF16, tag="ot", name=f"ot{qt}_{c}")
                nc.scalar.activation(out=ot[:], in_=op[:], func=AF.Copy, scale=1.0 / (WS * WS))
                of = out_pool.tile([128, 512], F32, tag="of", name=f"of{qt}_{c}")
                nc.vector.tensor_add(of[:], ot[:], resid_sb[:, qt, c * 512:(c + 1) * 512])
                nc.sync.dma_start(
                    out=out[qt * 128:(qt + 1) * 128, c * 512:(c + 1) * 512], in_=of
                )

    nc.finalize()
    return nc


_NC_CACHE = {}


def _get_nc():
    if "nc" not in _NC_CACHE:
        _NC_CACHE["nc"] = build()
    return _NC_CACHE["nc"]


def _fp8(x):
    return np.clip(x, -240.0, 240.0).astype(NP_FP8)


def make_in_maps(sources, queries, w_in, b_in, w_out, b_out):
    sources = np.asarray(sources, dtype=np.float32)
    queries = np.asarray(queries, dtype=np.float32)
    w_in = np.asarray(w_in, dtype=np.float32)
    b_in = np.asarray(b_in, dtype=np.float32)
    w_out = np.asarray(w_out, dtype=np.float32)
    b_out = np.asarray(b_out, dtype=np.float32)

    w_q, w_k, w_v = w_in[0:D], w_in[D:2 * D], w_in[2 * D:3 * D]
    b_q, b_v = b_in[0:D], b_in[2 * D:3 * D]
    bout_eff = b_out + w_out @ b_v

    def wprep(w):  # [dout, din] -> fp8 [128, KT, D] p-major of (w.T * WS)
        wt = np.ascontiguousarray(w.T) * WS
        return _fp8(wt.reshape(KT, 128, D).transpose(1, 0, 2))

    wk8 = wprep(w_k)
    wv8 = wprep(w_v)
    wq8 = wprep(w_q)
    wo8 = wprep(w_out)
    bq8 = (b_q / 8.0).reshape(MT, 128).transpose(1, 0).copy()

    in_maps = []
    for b in range(B):
        st = sources[b].T  # [D, N]
        src8 = _fp8(st.reshape(KT, 128, NW, 512).transpose(2, 1, 0, 3))
        qt = queries[b].T  # [D, Q]
        qry8 = _fp8(qt.reshape(KT, 128, Q).transpose(1, 0, 2))
        res = (queries[b] + bout_eff[None, :]).reshape(Q // 128, 128, D).transpose(1, 0, 2).copy()
        in_maps.append({
            "src8": src8, "qry8": qry8,
            "wk8": wk8, "wv8": wv8, "wq8": wq8, "wo8": wo8,
            "bq8": bq8, "resid": res,
        })
    return in_maps


def kernel(sources, queries, w_in, b_in, w_out, b_out, _trace=False):
    nc = _get_nc()
    in_maps = make_in_maps(sources, queries, w_in, b_in, w_out, b_out)
    res = run_bass_kernel_spmd(nc, in_maps, core_ids=list(range(N_CORES)), trace=_trace)
    out = np.stack([res.results[b]["out"] for b in range(B)], axis=0)
    if _trace:
        kernel.last_exec_time_ns = res.exec_time_ns
        kernel.last_results = res
    return out


# revision 18
# speedup vs baseline: 2.4405x; 1.0169x over previous
"""Trainium2 Bass kernel for nn_CrossAttentionLayer (B=8, N=2048, Q=256, D=1024, H=16).

Data-parallel over batch: 1 sample per NeuronCore, 8 cores, no collectives.

Math identities (host-side folds):
  - b_k dropped (constant shift along the softmax axis)
  - b_v folded through out_proj: bout_eff = b_out + W_o b_v (softmax rows sum to 1)
  - b_q and the 1/sqrt(HD) scale folded into the Q-projection eviction
  - exp computed as exp(s - 3): the e^-3 cancels in softmax normalization and
    keeps probs inside fp8-e4m3 normal range (max score ~5.5 -> e^2.5 ~ 12)

Precision plan (rel-err gate is 2e-2; residual dominates the output):
  - K/V/Q/out projections in fp8 e4m3 (weights host-scaled x32, unscaled at
    psum eviction); K and V use DoubleRow (2 k-tiles per matmul ~ 2x PE rate)
  - scores in bf16 with per-head-pair ROW TILING: head 2p on PE rows 0-63,
    head 2p+1 on rows 64-127, running concurrently (contraction = HD = 64)
  - probs in fp8 from ACT exp; attn@V in fp8 DoubleRow with a ones column
    appended to V (row 64 of the psum = softmax denominator, free)
  - normalization: batched reciprocal of denominators + PE broadcast (x32 to
    re-center fp8) + one DVE mul per pair

Engine budget per core: PE ~103us, ACT (exp only) ~73us, DVE (all psum
evictions) ~55us, DMA ~8.3MB in. ACT table + HAM warm-up during initial DMA.
"""

import numpy as np
import ml_dtypes
from contextlib import ExitStack

import concourse.bass as bass
import concourse.mybir as mybir
import concourse.tile as tile
from concourse import bacc
from concourse.bass_utils import run_bass_kernel_spmd

F32 = mybir.dt.float32
BF16 = mybir.dt.bfloat16
FP8 = mybir.dt.float8e4
AF = mybir.ActivationFunctionType
ALU = mybir.AluOpType
DR = mybir.MatmulPerfMode.DoubleRow

NP_FP8 = ml_dtypes.float8_e4m3
NP_BF16 = ml_dtypes.bfloat16

B, N, Q, D, H = 8, 2048, 256, 1024, 16
HD = D // H            # 64
KT = D // 128          # 8 contraction tiles
MT = D // 128          # 8 output tiles
NT = N // 128          # 16 token tiles
NW = N // 512          # 4 token windows (DMA + Kproj chunking)
PAIRS = H // 2         # 8 head pairs
WS = 32.0              # host weight pre-scale for fp8
N_CORES = 8


def build():
    nc = bacc.Bacc(None, target_bir_lowering=False)
    src8 = nc.declare_dram_parameter("src8", [NW, 128, KT, 512], FP8, isOutput=False)
    qry8 = nc.declare_dram_parameter("qry8", [128, KT, Q], FP8, isOutput=False)
    wk8 = nc.declare_dram_parameter("wk8", [128, KT, D], FP8, isOutput=False)
    wv8 = nc.declare_dram_parameter("wv8", [128, KT, D], FP8, isOutput=False)
    wq8 = nc.declare_dram_parameter("wq8", [128, KT, D], FP8, isOutput=False)
    wo8 = nc.declare_dram_parameter("wo8", [128, KT, D], FP8, isOutput=False)
    bq8 = nc.declare_dram_parameter("bq8", [128, MT], F32, isOutput=False)
    resid = nc.declare_dram_parameter("resid", [128, Q // 128, D], F32, isOutput=False)
    out = nc.declare_dram_parameter("out", [Q, D], F32, isOutput=True)

    with tile.TileContext(nc) as tc, ExitStack() as ctx:
        proj_ps = ctx.enter_context(tc.tile_pool(name="projps", bufs=2, space="PSUM"))
        sc_ps = ctx.enter_context(tc.tile_pool(name="scps", bufs=2, space="PSUM"))
        pso_ps = ctx.enter_context(tc.tile_pool(name="psops", bufs=2, space="PSUM"))

        big = ctx.enter_context(tc.tile_pool(name="big", bufs=1))
        exp_pool = ctx.enter_context(tc.tile_pool(name="expp", bufs=8))
        rb_pool = ctx.enter_context(tc.tile_pool(name="rbp", bufs=2))
        out_pool = ctx.enter_context(tc.tile_pool(name="outp", bufs=2))

        src_sb = big.tile([128, KT, NW, 512], FP8, tag="src")
        wk_sb = big.tile([128, KT, D], FP8, tag="wk")
        wv_sb = big.tile([128, KT, D], FP8, tag="wv")
        wq_sb = big.tile([128, KT, D], FP8, tag="wq")
        wo_sb = big.tile([128, KT, D], FP8, tag="wo")
        qry_sb = big.tile([128, KT, Q], FP8, tag="qry")
        bq_sb = big.tile([128, MT], F32, tag="bq")
        kt_sb = big.tile([128, MT, N], BF16, tag="kt")
        qt_sb = big.tile([128, MT, Q], BF16, tag="qt")
        v_sb = big.tile([128, NT, H, 66], FP8, tag="v")
        den_sb = big.tile([1, H, Q], BF16, tag="den")
        e32_sb = big.tile([1, 64], BF16, tag="e32")
        ao_sb = big.tile([128, MT, Q], BF16, tag="ao")
        ao8_sb = big.tile([128, MT, Q], FP8, tag="ao8")
        resid_sb = big.tile([128, Q // 128, D], F32, tag="res")
        warm_sb = big.tile([16, 512], BF16, tag="warm")
        warmo_sb = big.tile([16, 16], F32, tag="warmo")
        nb_sb = big.tile([128, 1], F32, tag="negbias")

        # ---- init: memsets (gpsimd), ACT exp-table warm, PE HAM warm ----
        nc.gpsimd.memset(warm_sb, 0.0)
        nc.vector.memset(warmo_sb, 0.0)
        nc.vector.memset(e32_sb, 1.0 / WS)  # denb = den/32 -> rb = 32/den
        nc.gpsimd.memset(v_sb[:, :, :, 64:65], 1.0)   # denominator ones column
        nc.gpsimd.memset(v_sb[:, :, :, 65:66], 0.0)   # padding
        nc.gpsimd.memset(nb_sb, -3.0)                 # exp re-centering bias
        # load the exp table set early (hides the ~2.7us ACT_TABLE_LOAD)
        nc.scalar.activation(out=warmo_sb[0:1, :], in_=warm_sb[0:1, 0:16], func=AF.Exp)
        # dummy matmuls to trip the PE HAM un-throttle during the DMA window
        for i in range(16):
            wp = proj_ps.tile([16, 512], F32, tag="proj", name=f"warm{i}")
            nc.tensor.matmul(wp[:], lhsT=warm_sb[:, 0:16], rhs=warm_sb[:], start=True, stop=True)

        # ---- DMA loads (sync queue), priority order ----
        nc.sync.dma_start(out=qry_sb, in_=qry8[:])
        nc.sync.dma_start(out=wq_sb, in_=wq8[:])
        nc.sync.dma_start(out=bq_sb, in_=bq8[:])
        for w in range(NW):
            nc.sync.dma_start(out=src_sb[:, :, w, :], in_=src8[w])
        nc.sync.dma_start(out=wk_sb, in_=wk8[:])
        nc.sync.dma_start(out=wv_sb, in_=wv8[:])
        nc.sync.dma_start(out=wo_sb, in_=wo8[:])
        nc.sync.dma_start(out=resid_sb, in_=resid[:])

        # ---- Q projection (plain fp8): qt = (psum/(WS*8)) + b_q/8 ----
        for m in range(MT):
            qp = proj_ps.tile([128, Q], F32, tag="proj", name=f"qp{m}")
            for k in range(KT):
                nc.tensor.matmul(
                    qp[:], lhsT=wq_sb[:, k, m * 128:(m + 1) * 128],
                    rhs=qry_sb[:, k, :], start=(k == 0), stop=(k == KT - 1),
                )
            nc.vector.tensor_scalar(
                out=qt_sb[:, m, :], in0=qp[:],
                scalar1=1.0 / (WS * 8.0), scalar2=bq_sb[:, m:m + 1],
                op0=ALU.mult, op1=ALU.add,
            )

        # ---- K projection (fp8 DoubleRow): kT[dout, tok] = W_k @ src^T ----
        def emit_kproj(m, w):
            kp = proj_ps.tile([128, 512], F32, tag="proj", name=f"kp{m}_{w}")
            for k in range(KT // 2):
                nc.tensor.matmul(
                    kp[:],
                    lhsT=wk_sb[:, 2 * k:2 * k + 2, m * 128:(m + 1) * 128],
                    rhs=src_sb[:, 2 * k:2 * k + 2, w, :],
                    start=(k == 0), stop=(k == KT // 2 - 1), perf_mode=DR,
                )
            nc.vector.tensor_scalar_mul(
                out=kt_sb[:, m, w * 512:(w + 1) * 512], in0=kp[:], scalar1=1.0 / WS
            )

        # ---- V projection (fp8 DoubleRow): v[tok, h, hd] = src @ W_v^T ----
        def emit_vproj(c, t):
            vp = proj_ps.tile([128, 512], F32, tag="proj", name=f"vp{c}_{t}")
            for k in range(KT // 2):
                nc.tensor.matmul(
                    vp[:],
                    lhsT=src_sb[:, 2 * k:2 * k + 2, t // 4, (t % 4) * 128:(t % 4) * 128 + 128],
                    rhs=wv_sb[:, 2 * k:2 * k + 2, c * 512:(c + 1) * 512],
                    start=(k == 0), stop=(k == KT // 2 - 1), perf_mode=DR,
                )
            nc.vector.tensor_scalar_mul(
                out=v_sb[:, t, c * 8:(c + 1) * 8, 0:64],
                in0=vp[:].rearrange("p (h d) -> p h d", h=8),
                scalar1=1.0 / WS,
            )

        # ---- attention: row-tiled scores + DR attn@V, emission-interleaved
        # with the K/V projections so ACT exp runs continuously ----
        expt = {}

        def emit_score_chunk(p, par, c):
            # 4 score matmuls + one exp for head 2p+par, n-tiles 4c..4c+3
            if (p, par) not in expt:
                expt[(p, par)] = exp_pool.tile(
                    [128, NT, Q], FP8, tag="exp", name=f"expt{p}_{par}"
                )
            po = par * 64
            sc = sc_ps.tile([128, 4, Q], F32, tag="sc", name=f"sc{p}_{c}_{par}")
            for j in range(4):
                nt = 4 * c + j
                nc.tensor.matmul(
                    sc[:, j, :],
                    lhsT=kt_sb[po:po + 64, p, nt * 128:(nt + 1) * 128],
                    rhs=qt_sb[po:po + 64, p, :],
                    start=True, stop=True,
                )
            nc.scalar.activation(
                out=expt[(p, par)][:, 4 * c:4 * c + 4, :], in_=sc[:],
                func=AF.Exp, bias=nb_sb[:],
            )

        def emit_scores_half(p, half):
            # half 0: chunks (e,0) (o,0) (e,1) (o,1); half 1: c = 2,3
            for c in (2 * half, 2 * half + 1):
                for par in range(2):
                    emit_score_chunk(p, par, c)

        def emit_attnv(p):
            for par in range(2):
                h = 2 * p + par
                pso = pso_ps.tile([65, Q], F32, tag="pso", name=f"pso{h}")
                for tt in range(NT // 2):
                    nc.tensor.matmul(
                        pso[:],
                        lhsT=v_sb[:, 2 * tt:2 * tt + 2, h, 0:65],
                        rhs=expt[(p, par)][:, 2 * tt:2 * tt + 2, :],
                        start=(tt == 0), stop=(tt == NT // 2 - 1), perf_mode=DR,
                    )
                nc.vector.tensor_copy(ao_sb[par * 64:par * 64 + 64, p, :], pso[0:64, :])
                nc.vector.tensor_copy(den_sb[:, h, :], pso[64:65, :])

        def emit_norm(p):
            denb = proj_ps.tile([128, Q], F32, tag="proj", name=f"denb{p}")
            for par in range(2):
                nc.tensor.matmul(
                    denb[par * 64:par * 64 + 64, :], lhsT=e32_sb[:],
                    rhs=den_sb[:, 2 * p + par, :], start=True, stop=True,
                )
            rb = rb_pool.tile([128, Q], F32, tag="rb", name=f"rb{p}")
            nc.vector.reciprocal_approx_fast(out=rb[:], in_=denb[:])
            nc.vector.tensor_mul(ao8_sb[:, p, :], ao_sb[:, p, :], rb[:])

        # fine-grained interleave: score half-blocks (PE-light, feeds ACT)
        # alternate with projection / attn@V / normalize blocks (PE-heavy),
        # so the in-order PE queue never waits on ACT psum recycling.
        def K(m):
            for w in range(NW):
                emit_kproj(m, w)

        def V(c, lo, hi):
            for t in range(lo, hi):
                emit_vproj(c, t)

        K(0); K(1)
        emit_scores_half(0, 0); K(2)
        emit_scores_half(0, 1); K(3)
        emit_scores_half(1, 0); K(4)
        emit_scores_half(1, 1); K(5)
        emit_scores_half(2, 0); V(0, 0, 8)
        emit_scores_half(2, 1); V(0, 8, 16)
        emit_scores_half(3, 0); K(6)
        emit_scores_half(3, 1); emit_attnv(0)
        emit_scores_half(4, 0); K(7)
        emit_scores_half(4, 1); emit_attnv(1)
        emit_scores_half(5, 0); V(1, 0, 8)
        emit_scores_half(5, 1); emit_attnv(2); emit_norm(0)
        emit_scores_half(6, 0); V(1, 8, 16)
        emit_scores_half(6, 1); emit_attnv(3); emit_norm(1)
        emit_scores_half(7, 0); emit_attnv(4); emit_norm(2)
        emit_scores_half(7, 1); emit_attnv(5); emit_norm(3)
        emit_attnv(6); emit_norm(4); emit_norm(5)
        emit_attnv(7); emit_norm(6); emit_norm(7)

        # ---- out projection (fp8 DoubleRow) + residual ----
        for qt in range(Q // 128):
            for c in range(2):
                op = proj_ps.tile([128, 512], F32, tag="proj", name=f"op{qt}_{c}")
                for m in range(MT // 2):
                    nc.tensor.matmul(
                        op[:],
                        lhsT=ao8_sb[:, 2 * m:2 * m + 2, qt * 128:(qt + 1) * 128],
                        rhs=wo_sb[:, 2 * m:2 * m + 2, c * 512:(c + 1) * 512],
                        start=(m == 0), stop=(m == MT // 2 - 1), perf_mode=DR,
                    )
                ot = out_pool.tile([128, 512], BF16, tag="ot", name=f"ot{qt}_{c}")
                nc.scalar.activation(out=ot[:], in_=op[:], func=AF.Copy, scale=1.0 / (WS * WS))
                of = out_pool.tile([128, 512], F32, tag="of", name=f"of{qt}_{c}")
                nc.vector.tensor_add(of[:], ot[:], resid_sb[:, qt, c * 512:(c + 1) * 512])
                nc.sync.dma_start(
                    out=out[qt * 128:(qt + 1) * 128, c * 512:(c + 1) * 512], in_=of
                )

    nc.finalize()
    return nc


_NC_CACHE = {}


def _get_nc():
    if "nc" not in _NC_CACHE:
        _NC_CACHE["nc"] = build()
    return _NC_CACHE["nc"]


def _fp8(x):
    return np.clip(x, -240.0, 240.0).astype(NP_FP8)


def make_in_maps(sources, queries, w_in, b_in, w_out, b_out):
    sources = np.asarray(sources, dtype=np.float32)
    queries = np.asarray(queries, dtype=np.float32)
    w_in = np.asarray(w_in, dtype=np.float32)
    b_in = np.asarray(b_in, dtype=np.float32)
    w_out = np.asarray(w_out, dtype=np.float32)
    b_out = np.asarray(b_out, dtype=np.float32)

    w_q, w_k, w_v = w_in[0:D], w_in[D:2 * D], w_in[2 * D:3 * D]
    b_q, b_v = b_in[0:D], b_in[2 * D:3 * D]
    bout_eff = b_out + w_out @ b_v

    def wprep(w):  # [dout, din] -> fp8 [128, KT, D] p-major of (w.T * WS)
        wt = np.ascontiguousarray(w.T) * WS
        return _fp8(wt.reshape(KT, 128, D).transpose(1, 0, 2))

    wk8 = wprep(w_k)
    wv8 = wprep(w_v)
    wq8 = wprep(w_q)
    wo8 = wprep(w_out)
    bq8 = (b_q / 8.0).reshape(MT, 128).transpose(1, 0).copy()

    in_maps = []
    for b in range(B):
        st = sources[b].T  # [D, N]
        src8 = _fp8(st.reshape(KT, 128, NW, 512).transpose(2, 1, 0, 3))
        qt = queries[b].T  # [D, Q]
        qry8 = _fp8(qt.reshape(KT, 128, Q).transpose(1, 0, 2))
        res = (queries[b] + bout_eff[None, :]).reshape(Q // 128, 128, D).transpose(1, 0, 2).copy()
        in_maps.append({
            "src8": src8, "qry8": qry8,
            "wk8": wk8, "wv8": wv8, "wq8": wq8, "wo8": wo8,
            "bq8": bq8, "resid": res,
        })
    return in_maps


def kernel(sources, queries, w_in, b_in, w_out, b_out, _trace=False):
    nc = _get_nc()
    in_maps = make_in_maps(sources, queries, w_in, b_in, w_out, b_out)
    res = run_bass_kernel_spmd(nc, in_maps, core_ids=list(range(N_CORES)), trace=_trace)
    out = np.stack([res.results[b]["out"] for b in range(B)], axis=0)
    if _trace:
        kernel.last_exec_time_ns = res.exec_time_ns
        kernel.last_results = res
    return out
